# revision 2
# baseline (speedup 1.0000x reference)
"""CRF loss (forward-algorithm partition function minus gold score, batch mean)
on 8 Trainium2 NeuronCores.  v2.

Strategy: pure data parallel over batch (512 -> 64 per core) + 64-way
sequence split per core (chains of L=16 steps, DELTA=2 warmup from uniform,
stitched with column-sum ratios, computed host-side from shipped states).

Key structure:
  - exp(feats) is precomputed on the host and shipped as fp8 in ROUND-MAJOR
    order (DMA order == consumption order), so the device does no exp work.
  - both 64-tag halves ride in one 128-partition op per path: the matmul
    stationary is blockdiag(M, M); PE cost is per output column only.
  - the per-round elementwise multiply is split by column range across two
    engine paths with separate state/PSUM tiles (separate tiles so the tile
    framework never serializes their writes, and the column split is sized
    so both paths run at the SAME pace -- the in-order PE queue chains them
    together, so unequal paces would drag both to the slower one):
      path A (652 cols): DVE tensor_tensor straight from PSUM
      path C (372 cols): Act copies PSUM->SBUF bf16, GPSIMD multiplies
  - stitching data: the warmup-end states (Act-copied to spare SBUF at
    round DELTA) and the final states are DMA'd out whole; the host does
    column sums, ln, the chain-63 stop terminal, and the stitch in f64.
  - gold emit score: host gathers feats[b,s,tag] in f32; device reduces.
"""

import numpy as np
import ml_dtypes
from contextlib import ExitStack

import concourse.bass as bass
import concourse.tile as tile
from concourse import bacc, mybir
from concourse.bass_utils import run_bass_kernel_spmd

F32 = mybir.dt.float32
F8 = mybir.dt.float8e4
BF16 = mybir.dt.bfloat16

B, S, T = 512, 1024, 64
NCORES = 8
BS = B // NCORES          # 64 batches per core
START_TAG, STOP_TAG = 62, 63
CSHIFT = 5.1              # per-step constant log shift folded into M

C = 64                    # chains per batch (sequence segments)
L = S // C                # 16 steps per chain
DELTA = 1                 # warmup steps per chain
R = L + DELTA             # 18 rounds
NSTR = 2                  # software-pipelined column streams
CPS = C // (2 * NSTR)     # chains per (set, stream) = 16
W2 = CPS * BS             # 1024 columns per stream

A2 = 652                  # path A columns (DVE); need not align to chains
C2 = W2 - A2              # path C columns (Act copy + GPSIMD tensor_tensor)
QA = [512, A2 - 512]      # PSUM tile widths for path A (bank limit 512 f32)


def chain_of(seti, u, cl):
    """(partition set, stream, local chain 0..15) -> global chain index."""
    return seti * (C // 2) + u * CPS + cl


def crf_kernel(ctx: ExitStack, tc: tile.TileContext, outs, ins):
    nc = tc.nc
    (hA0_o, hA1_o, hC0_o, hC1_o, fA0_o, fA1_o, fC0_o, fC1_o,
     esum_o, tsum_o) = outs
    (w128_i, init64_i, ef8_i, efC_i, g32_i, table_i, counts_i) = ins
    hA_o, hC_o = (hA0_o, hA1_o), (hC0_o, hC1_o)
    fA_o, fC_o = (fA0_o, fA1_o), (fC0_o, fC1_o)

    const = ctx.enter_context(tc.tile_pool(name="const", bufs=1))
    qp = ctx.enter_context(tc.tile_pool(name="qp", bufs=1, space="PSUM"))
    sbp = ctx.enter_context(tc.tile_pool(name="sbp", bufs=2))

    # ---- DMA plan: SP queue carries w128 + the round-major ef stream
    # (+init64 slipped in early); table/counts/g32 ride the DVE queue
    # before the loop; Act stays clear for the round-DELTA reset. ----
    w128 = const.tile([128, 128], BF16)
    nc.sync.dma_start(w128[:, :], w128_i[:, :])

    ef8 = const.tile([128, R * NSTR * A2], F8)
    efC = const.tile([128, R * NSTR * C2], BF16)
    init64 = const.tile([64, BS], BF16)
    g32 = const.tile([128, 512], F32)
    CHUNKS = [1, 3, 6, R - 10]  # rounds per DMA chunk (small first); each
    r0 = 0                      # chunk is ONE dma (HWDGE gen is ~625ns each)
    for ci, ch in enumerate(CHUNKS):
        r1 = min(R, r0 + ch)
        if r1 > r0:
            nc.sync.dma_start(ef8[:, r0 * NSTR * A2:r1 * NSTR * A2],
                              ef8_i[:, r0 * NSTR * A2:r1 * NSTR * A2])
            nc.sync.dma_start(efC[:, r0 * NSTR * C2:r1 * NSTR * C2],
                              efC_i[:, r0 * NSTR * C2:r1 * NSTR * C2])
        if ci == 0:
            nc.sync.dma_start(init64[:, :], init64_i[:, :])
        if ci == 2:
            nc.sync.dma_start(g32[:, :], g32_i[:, :])
        r0 = r1

    # table/counts are only needed by the tail tsum; last on the SP queue
    table = const.tile([128, 256], F32)
    nc.sync.dma_start(table[:, :], table_i[:, :])
    counts = const.tile([128, 256], BF16)
    nc.sync.dma_start(counts[:, :], counts_i[:, :])

    # ---- state tiles: [stream][parity], per path ----
    stA = [[const.tile([128, A2], BF16, tag=f"stA{u}_{p}", name=f"stA{u}_{p}")
            for p in range(2)] for u in range(NSTR)]
    stC = [[const.tile([128, C2], BF16, tag=f"stC{u}_{p}", name=f"stC{u}_{p}")
            for p in range(2)] for u in range(NSTR)]
    for u in range(NSTR):
        nc.vector.memset(stA[u][0][:, :], 1.0 / T)
        nc.gpsimd.memset(stC[u][0][:, :], 1.0 / T)

    # h capture buffers (bf16 copies of the warmup-end states)
    hcA = [const.tile([128, A2], BF16, tag=f"hcA{u}", name=f"hcA{u}")
           for u in range(NSTR)]
    hcC = [const.tile([128, C2], BF16, tag=f"hcC{u}", name=f"hcC{u}")
           for u in range(NSTR)]

    def a_in(u, r):
        # tile holding the state ENTERING round r; round DELTA's input is
        # the hc tile, which round DELTA-1 wrote directly -> the h capture
        # costs nothing and is never overwritten.
        return hcA[u] if r == DELTA else stA[u][r % 2]

    def c_in(u, r):
        return hcC[u] if r == DELTA else stC[u][r % 2]

    def mroundA(u, r):
        st_in, st_out = a_in(u, r), a_in(u, r + 1)
        if r == DELTA and u == 0:
            # reset chain 0 to the exact e_start before its round-DELTA matmul
            nc.scalar.copy(hcA[0][0:64, 0:BS], init64[:, :])
        qA = [qp.tile([128, w], F32, tag=f"qA{u}_{i}", name=f"qA{u}_{i}")
              for i, w in enumerate(QA)]
        o = 0
        for i, w in enumerate(QA):
            nc.tensor.matmul(qA[i][:, :], w128[:, :], st_in[:, o:o + w])
            o += w
        cb = (r * NSTR + u) * A2
        o = 0
        for i, w in enumerate(QA):
            nc.vector.tensor_tensor(st_out[:, o:o + w], qA[i][:, :],
                                    ef8[:, cb + o:cb + o + w],
                                    op=mybir.AluOpType.mult)
            o += w

    def mroundC(u, r):
        # GPSIMD cannot touch PSUM (walrus/birverifier enforces this), so
        # the idle Act engine stages q into SBUF bf16 first.
        sc_in, sc_out = c_in(u, r), c_in(u, r + 1)
        qC = qp.tile([128, C2], F32, tag=f"qC{u}")
        nc.tensor.matmul(qC[:, :], w128[:, :], sc_in[:, :])
        sq = sbp.tile([128, C2], BF16, tag=f"sbqC{u}")
        nc.scalar.copy(sq[:, :], qC[:, :])
        cb = (r * NSTR + u) * C2
        nc.gpsimd.tensor_tensor(sc_out[:, :], sq[:, :],
                                efC[:, cb:cb + C2],
                                op=mybir.AluOpType.mult)

    # ---- main loop: A issued LEAD rounds ahead of C so the static
    # scheduler's priority order never parks an unready C matmul in front
    # of a ready A matmul (engines execute their queues in order). ----
    LEAD = 1
    fp = R % 2  # parity holding the final states
    for l in range(NSTR * (R + LEAD)):
        u = l % NSTR
        ra = l // NSTR
        r = ra - LEAD
        if ra < R:
            with tc.high_priority(offset=1 << 20):
                mroundA(u, ra)
        if r >= 0:
            mroundC(u, r)
        if r == DELTA + 4 and u == 0:
            # gold emit sum on the idle Act engine (activation accumulate)
            dump = const.tile([128, 512], F32)
            esum = const.tile([128, 1], F32)
            nc.scalar.activation(dump[:, :], g32[:, :],
                                 mybir.ActivationFunctionType.Identity,
                                 accum_out=esum[:, :])
            nc.scalar.dma_start(esum_o[:, :], esum[:, :])
        if r == R - 3 and u == 0:
            # ship the h captures now that the ef input stream has drained
            for v in range(NSTR):
                nc.scalar.dma_start(hA_o[v][:, :], hcA[v][:, :])
                nc.scalar.dma_start(hC_o[v][:, :], hcC[v][:, :])
        if r == R - 1:
            # final-state DMAs issued as soon as this stream's last multiply
            # is in flight; dispatch/DGE overlap the other stream's tail.
            nc.scalar.dma_start(fA_o[u][:, :], stA[u][fp][:, :])
            nc.scalar.dma_start(fC_o[u][:, :], stC[u][fp][:, :])

    # gold transition sum on DVE once its loop work is done
    tsc = const.tile([128, 256], F32)
    tsum = const.tile([128, 1], F32)
    nc.vector.scalar_tensor_tensor(tsc[:, :], table[:, :], 1.0,
                                   counts[:, :],
                                   op0=mybir.AluOpType.mult,
                                   op1=mybir.AluOpType.mult,
                                   accum_out=tsum[:, :])
    nc.sync.dma_start(tsum_o[:, :], tsum[:, :])


def build():
    nc = bacc.Bacc("TRN2", target_bir_lowering=False, debug=False)
    ins_spec = [
        ("w128", [128, 128], BF16),
        ("init64", [64, BS], BF16),
        ("ef8", [128, R * NSTR * A2], F8),
        ("efC", [128, R * NSTR * C2], BF16),
        ("g32", [128, 512], F32),
        ("table", [128, 256], F32),
        ("counts", [128, 256], BF16),
    ]
    outs_spec = [
        ("hA0", [128, A2], BF16), ("hA1", [128, A2], BF16),
        ("hC0", [128, C2], BF16), ("hC1", [128, C2], BF16),
        ("fA0", [128, A2], BF16), ("fA1", [128, A2], BF16),
        ("fC0", [128, C2], BF16), ("fC1", [128, C2], BF16),
        ("esum", [128, 1], F32),
        ("tsum", [128, 1], F32),
    ]
    ins = [nc.declare_dram_parameter(n, s, d, isOutput=False).ap()
           for n, s, d in ins_spec]
    outs = [nc.declare_dram_parameter(n, s, d, isOutput=True).ap()
            for n, s, d in outs_spec]
    with tile.TileContext(nc) as tc:
        with ExitStack() as ctx:
            crf_kernel(ctx, tc, outs, ins)
    nc.compile()
    return nc


def host_prep(feats, transitions, tags, mask):
    """Build the 8 per-core input maps."""
    assert feats.shape == (B, S, T) and transitions.shape == (T, T)
    mask_arr = np.asarray(mask)
    assert np.all(mask_arr == 1), "kernel assumes an all-ones mask"
    feats = np.asarray(feats, dtype=np.float32)
    transitions = np.asarray(transitions, dtype=np.float32)
    tags = np.asarray(tags).astype(np.int64)

    bf = ml_dtypes.bfloat16
    f8 = ml_dtypes.float8_e4m3fn

    mt = np.exp(transitions.T - CSHIFT).astype(bf)       # [p, n]
    w128 = np.zeros((128, 128), bf)
    w128[0:64, 0:64] = mt
    w128[64:128, 64:128] = mt
    init64 = np.zeros((64, BS), bf)
    init64[START_TAG, :] = 1.0

    tflat = transitions.reshape(4096)
    table = np.zeros((128, 256), np.float32)
    p_ = np.arange(128)
    table[:, :] = tflat[(p_[:, None] % 16) + 16 * np.arange(256)[None, :]]

    in_maps = []
    for core in range(NCORES):
        b0 = core * BS
        fb = feats[b0:b0 + BS]                           # (64, 1024, 64) f32
        tg = tags[b0:b0 + BS]

        ef8 = np.exp(fb).astype(f8)                      # (b, s, n) fp8

        # round-major ef: [p = set*64+tag, r, u, local col = cl*64+b]
        ef_rm = np.ones((128, R, NSTR, W2), f8)
        ks = np.arange(C)
        for r in range(R):
            svec = ks * L - DELTA + r                    # (C,)
            for seti in range(2):
                for u in range(NSTR):
                    cls = np.arange(CPS)
                    kk = chain_of(seti, u, cls)          # (CPS,)
                    ss = svec[kk]                        # (CPS,)
                    ok = ss >= 0
                    blk = ef8[:, ss[ok], :].transpose(2, 1, 0)  # (n, cl, b)
                    dst3 = ef_rm[seti * 64:seti * 64 + 64, r, u].reshape(
                        64, CPS, BS)
                    dst3[:, ok, :] = blk
        v = ef_rm.reshape(128, R * NSTR, W2)
        ef8v = np.ascontiguousarray(v[:, :, 0:A2])
        ef8 = ef8v.reshape(128, R * NSTR * A2)
        # same fp8 VALUES, bf16 container (GPSIMD-friendly dtype)
        efC = np.ascontiguousarray(v[:, :, A2:W2]).astype(bf)
        efC = efC.reshape(128, R * NSTR * C2)

        # gold emit gather (f32, exact)
        g = np.take_along_axis(fb, tg[:, :, None], axis=2)[:, :, 0]
        g32 = np.ascontiguousarray(g.reshape(64, 1024).T.reshape(128, 512))

        # transition-pair histogram
        cur = np.concatenate([tg, np.full((BS, 1), STOP_TAG, np.int64)], 1)
        prev = np.concatenate([np.full((BS, 1), START_TAG, np.int64), tg], 1)
        lin = (cur * T + prev).reshape(-1)
        cnt = np.bincount(lin, minlength=4096)
        assert cnt.max() < 256
        counts = np.zeros((128, 256), bf)
        counts[0:16, :] = cnt.reshape(256, 16).T

        in_maps.append({
            "w128": w128, "init64": init64,
            "ef8": ef8, "efC": efC, "g32": g32,
            "table": table, "counts": counts,
        })
    return in_maps


def _stitch_core(res, transitions):
    """Host-side column sums + ln + stitch for one core -> per-batch fwd."""
    # states: [128, cols] bf16; parts = (set, tag); cols = (cl, b) per path
    hs = np.concatenate([res["hA0"], res["hC0"], res["hA1"], res["hC1"]],
                        axis=1).astype(np.float64)
    fs = np.concatenate([res["fA0"], res["fC0"], res["fA1"], res["fC1"]],
                        axis=1).astype(np.float64)

    def sums(x):
        # [128, NSTR*W2] -> [2 sets, NSTR, CPS, BS] per-chain column sums
        y = x.reshape(2, 64, NSTR, CPS, BS)
        return y.sum(axis=1)

    h = sums(hs)   # [2, NSTR, CPS, BS]
    t = sums(fs)
    lnh = np.log(h).sum(axis=(0, 1, 2))      # chain0's h == 1 -> ln 0
    # t summed over all chains except the global last (set1, u1, cl CPS-1)
    lnt = np.log(t).sum(axis=(0, 1, 2)) - np.log(t[1, 1, CPS - 1])
    # stop-weighted terminal for the last chain
    stop = np.exp(transitions[STOP_TAG, :].astype(np.float64))
    fC1 = res["fC1"].astype(np.float64)      # [128, C2]
    last = fC1[64:128, C2 - BS:C2]           # [64 tags, BS] last chain
    lnq = np.log(stop @ last)
    return lnt + lnq - lnh + S * CSHIFT


def host_finish(results, transitions):
    fwd_total = 0.0
    gold_total = 0.0
    for res in results:
        fwd_total += float(_stitch_core(res, transitions).sum())
        gold_total += float(res["esum"].astype(np.float64).sum())
        gold_total += float(res["tsum"].astype(np.float64).sum())
    return np.asarray((fwd_total - gold_total) / B, dtype=np.float32)


_NC = None


def kernel(feats, transitions, tags, mask):
    global _NC
    if _NC is None:
        _NC = build()
    transitions = np.asarray(transitions, dtype=np.float32)
    in_maps = host_prep(feats, transitions, tags, mask)
    res = run_bass_kernel_spmd(_NC, in_maps, list(range(NCORES)))
    return host_finish(res.results, transitions)


if __name__ == "__main__":
    import reference
    inp = reference.setup_inputs()
    out = kernel(**{k: np.asarray(v) for k, v in inp.items()})
    print("kernel loss:", out)
